# revision 2
# baseline (speedup 1.0000x reference)
"""MultiHeadPool Trainium2 kernel, v6.

bf16 "d-world" formulation (see kernel_v5) with DMA restructuring:
  - CH=32 t-pairs per chunk: fewer, 1MB-sized input DMAs
  - input chunk DMAs on the sync HWDGE ring, pre-transposed X^T reads on the
    scalar HWDGE ring, output stores on the gpsimd SWDGE ring (3 queues)
  - DMA_XT on odd chunks so chunk 0 starts PE work without waiting for xbt

Per chunk g of CH=32 t-pairs:
  chunk [jn=128, CH, 128] bf16  <- plain DMA      (mm2 operand)
  xt    [d=128, CH, 128] bf16   <- DRAM or PE     (mm1 operand)
  mm1:   L_c = xt_c.T @ qt -> [jn, 7] PSUM
  exp -> E [128, CH, 14] bf16 block structure (dead quadrants zeroed once)
  mm2:   lhsT=chunk_c, rhs=E_c -> ctx^T [d, 14] PSUM bank 0 of cxb
  denom: lhsT=ones, rhs=E -> [128, CH*14] at bank 1 of cxb (bcast over parts)
  recip_approx + multiply -> st fp32; DMA out
"""

import sys

for p in ("/opt/trn_rl_repo", "/root/.axon_site/_ro/trn_rl_repo"):
    if p not in sys.path:
        sys.path.append(p)

from contextlib import ExitStack

import numpy as np
import ml_dtypes

import concourse.bacc as bacc
import concourse.tile as tile
from concourse import mybir
from concourse.bass_utils import run_bass_kernel_spmd

B, N, T, D, H = 8, 64, 512, 128, 7
CH = 32               # t-pairs per chunk
NG = (T // 2) // CH   # 8 chunks per batch
E2 = 2 * H            # 14
CXW = 1024            # cxb tile width: ctx at cols 0:448, denom at 512:960
F32 = mybir.dt.float32
BF16 = mybir.dt.bfloat16

DMA_XT = [g % 2 == 1 for g in range(NG)]

_CACHE = {}


def _body(ctx, tc, xb, xbt, qt, ident, ob):
    nc = tc.nc

    singles = ctx.enter_context(tc.tile_pool(name="singles", bufs=1))
    chunks = ctx.enter_context(tc.tile_pool(name="chunks", bufs=8))
    xtsp = ctx.enter_context(tc.tile_pool(name="xts", bufs=8))
    xtp = ctx.enter_context(tc.tile_pool(name="xtp", bufs=2, space="PSUM"))
    ltp = ctx.enter_context(tc.tile_pool(name="ltp", bufs=2, space="PSUM"))
    cxp = ctx.enter_context(tc.tile_pool(name="cxp", bufs=2, space="PSUM"))
    rp = ctx.enter_context(tc.tile_pool(name="rp", bufs=4))
    stg = ctx.enter_context(tc.tile_pool(name="stg", bufs=4))

    qt_sb = singles.tile([D, H], BF16)
    nc.sync.dma_start(out=qt_sb[:], in_=qt[:])
    id_sb = singles.tile([D, D], BF16)
    nc.sync.dma_start(out=id_sb[:], in_=ident[:])
    ones_sb = singles.tile([D, D], BF16)
    nc.vector.memset(ones_sb[:], 1.0)

    e_buf = singles.tile([128, 4, CH, E2], BF16)
    nc.gpsimd.memset(e_buf[64:128, :, :, 0:H], 0.0)
    nc.gpsimd.memset(e_buf[0:64, :, :, H:E2], 0.0)

    kx = 0  # index into xbt
    for g in range(NG):
        chunk = chunks.tile([128, CH, D], BF16)
        nc.sync.dma_start(out=chunk[:], in_=xb[:, CH * g: CH * (g + 1), :])

        xt = xtsp.tile([128, CH, D], BF16)
        if DMA_XT[g]:
            nc.scalar.dma_start(out=xt[:], in_=xbt[kx])
            kx += 1
        else:
            for q4 in range(CH // 4):
                xtb = xtp.tile([128, 4, D], BF16)
                for i in range(4):
                    c = q4 * 4 + i
                    nc.tensor.transpose(xtb[:, i, :], chunk[:, c, :], id_sb[:])
                if q4 % 2 == 0:
                    nc.vector.tensor_copy(xt[:, q4 * 4: q4 * 4 + 4, :], xtb[:])
                else:
                    nc.scalar.copy(xt[:, q4 * 4: q4 * 4 + 4, :], xtb[:])

        ltb = ltp.tile([128, CH * H], F32)
        for c in range(CH):
            nc.tensor.matmul(
                ltb[:, H * c: H * (c + 1)],
                lhsT=xt[:, c, :],
                rhs=qt_sb[:],
                start=True, stop=True,
            )

        s = g % 4
        lt3 = ltb.rearrange("p (c h) -> p c h", h=H)
        nc.scalar.activation(
            out=e_buf[0:64, s, :, 0:H], in_=lt3[0:64],
            func=mybir.ActivationFunctionType.Exp,
        )
        nc.scalar.activation(
            out=e_buf[64:128, s, :, H:E2], in_=lt3[64:128],
            func=mybir.ActivationFunctionType.Exp,
        )

        cxb = cxp.tile([128, CXW], F32)
        for c in range(CH):
            nc.tensor.matmul(
                cxb[:, E2 * c: E2 * (c + 1)],
                lhsT=chunk[:, c, :],
                rhs=e_buf[:, s, c, :],
                start=True, stop=True,
            )
        nc.tensor.matmul(
            cxb[:, 512: 512 + CH * E2],
            lhsT=ones_sb[:],
            rhs=e_buf[:, s, :, :],
            start=True, stop=True,
        )
        rr = rp.tile([128, CH * E2], F32)
        nc.vector.reciprocal_approx_fast(out=rr[:], in_=cxb[:, 512: 512 + CH * E2])
        st = stg.tile([128, CH * E2], F32)
        nc.vector.scalar_tensor_tensor(
            out=st[:], in0=cxb[:, 0: CH * E2], scalar=1.0, in1=rr[:],
            op0=mybir.AluOpType.mult, op1=mybir.AluOpType.mult,
        )
        nc.gpsimd.dma_start(out=ob[g], in_=st[:])


def _build():
    nxt = sum(DMA_XT)
    nc = bacc.Bacc("TRN2", target_bir_lowering=False, debug=False)
    xb = nc.dram_tensor("xb", [128, T // 2, D], BF16, kind="ExternalInput")
    xbt = nc.dram_tensor("xbt", [nxt, 128, CH, 128], BF16, kind="ExternalInput")
    qt = nc.dram_tensor("qt", [D, H], BF16, kind="ExternalInput")
    ident = nc.dram_tensor("ident", [D, D], BF16, kind="ExternalInput")
    ob = nc.dram_tensor("ob", [NG, 128, CH * E2], F32, kind="ExternalOutput")
    with tile.TileContext(nc) as tc:
        with ExitStack() as ctx:
            _body(ctx, tc, xb[:], xbt[:], qt[:], ident[:], ob[:])
    nc.compile()
    return nc


def get_nc():
    if "nc" not in _CACHE:
        _CACHE["nc"] = _build()
    return _CACHE["nc"]


def prep_input(others_b):
    """others[b] (N,T,D) fp32 -> (xb [(j n), tp, d], xbt [k, d, c, jn]) bf16."""
    v = others_b.reshape(N, T // 2, 2, D).astype(ml_dtypes.bfloat16)
    x = np.empty((128, T // 2, D), dtype=ml_dtypes.bfloat16)
    x[0:64] = v[:, :, 0, :]
    x[64:128] = v[:, :, 1, :]
    xs = x.reshape(128, NG, CH, D)
    sel = [g for g in range(NG) if DMA_XT[g]]
    xt = np.ascontiguousarray(xs[:, sel].transpose(1, 3, 2, 0))  # k, d, c, jn
    return x, xt


def unpack_output(ob_raw):
    """(NG, 128, CH*E2) -> (T, H, D); t = 2(CH g + c) + j, col i = 7 j + h."""
    s = ob_raw.reshape(NG, 128, CH, 2, H)          # g, d, c, j, h
    return np.ascontiguousarray(
        s.transpose(0, 2, 3, 4, 1).reshape(T, H, D)
    )


def kernel(ego=None, others=None, queries=None, _trace=False, **_unused):
    others = np.asarray(others, dtype=np.float32)
    queries = np.asarray(queries, dtype=np.float32)
    scale = float(queries.shape[-1]) ** -0.5
    qt_bf = np.ascontiguousarray(queries.T * scale).astype(ml_dtypes.bfloat16)
    eye_bf = np.eye(D, dtype=ml_dtypes.bfloat16)

    nc = get_nc()
    in_maps = []
    for b in range(B):
        x, xt = prep_input(others[b])
        in_maps.append({"xb": x, "xbt": xt, "qt": qt_bf, "ident": eye_bf})
    res = run_bass_kernel_spmd(nc, in_maps, core_ids=list(range(B)), trace=_trace)
    _CACHE["last_results"] = res
    out = np.empty((B, T, H, D), dtype=np.float32)
    for b in range(B):
        out[b] = unpack_output(res.results[b]["ob"])
    return out


# revision 3
# speedup vs baseline: 1.0719x; 1.0719x over previous
"""MultiHeadPool Trainium2 kernel.

Per-core computation (batch b of 8, one per NeuronCore):
  X = others[b]       (N=64, T=512, D=128)
  L = X . qT * scale  -> softmax over n -> ctx = W . X   (T, H, D)

Layout: t-pairs stacked on partitions, xb[(j n), tp, d] = others[n, 2tp+j, d],
host-cast to bf16 (all matmuls bf16 with fp32 PSUM accumulation; rel err vs
the fp32 reference ~4e-3, gate is 2e-2).

Key structure:
  - "d-world" mm2: lhsT=chunk gives ctx TRANSPOSED [d, 14] per t-pair with
    zero padding waste, and a ones-lhsT matmul broadcasts the softmax
    denominators across all 128 partitions for free
  - X^T (mm1 operand): odd chunks read pre-transposed from a second host
    copy in DRAM (plain contiguous DMA), even chunks via PE transpose-mode +
    DVE/ACT relay -- balances the DMA stream against PE time
  - CH=32 t-pairs per chunk: 1MB input DMAs; chunk DMAs on the sync HWDGE
    ring, X^T reads on the scalar HWDGE ring, output stores on the gpsimd
    SWDGE ring (3 independent queues)
  - all chunks SBUF-resident (bufs=8) so input DMA free-runs ahead of compute

Per chunk g of CH=32 t-pairs:
  chunk [jn=128, CH, 128] bf16  <- plain DMA      (mm2 operand)
  xt    [d=128, CH, 128] bf16   <- DRAM or PE     (mm1 operand)
  mm1:   L_c = xt_c.T @ qt -> [jn, 7] PSUM
  exp -> E [128, CH, 14] bf16 block structure (dead quadrants zeroed once)
  mm2:   lhsT=chunk_c, rhs=E_c -> ctx^T [d, 14] PSUM bank 0 of cxb
  denom: lhsT=ones, rhs=E -> [128, CH*14] at bank 1 of cxb (bcast over parts)
  recip_approx + multiply -> st fp32; DMA out
"""

import sys

for p in ("/opt/trn_rl_repo", "/root/.axon_site/_ro/trn_rl_repo"):
    if p not in sys.path:
        sys.path.append(p)

from contextlib import ExitStack

import numpy as np
import ml_dtypes

import concourse.bacc as bacc
import concourse.tile as tile
from concourse import mybir
from concourse.bass_utils import run_bass_kernel_spmd

B, N, T, D, H = 8, 64, 512, 128, 7
CH = 32               # t-pairs per chunk
NG = (T // 2) // CH   # 8 chunks per batch
E2 = 2 * H            # 14
CXW = 1024            # cxb tile width: ctx at cols 0:448, denom at 512:960
F32 = mybir.dt.float32
BF16 = mybir.dt.bfloat16

DMA_XT = [g % 2 == 1 for g in range(NG)]

_CACHE = {}


def _body(ctx, tc, xb, xbt, qt, ident, ob):
    nc = tc.nc

    singles = ctx.enter_context(tc.tile_pool(name="singles", bufs=1))
    chunks = ctx.enter_context(tc.tile_pool(name="chunks", bufs=8))
    xtsp = ctx.enter_context(tc.tile_pool(name="xts", bufs=8))
    xtp = ctx.enter_context(tc.tile_pool(name="xtp", bufs=2, space="PSUM"))
    ltp = ctx.enter_context(tc.tile_pool(name="ltp", bufs=2, space="PSUM"))
    cxp = ctx.enter_context(tc.tile_pool(name="cxp", bufs=2, space="PSUM"))
    rp = ctx.enter_context(tc.tile_pool(name="rp", bufs=4))
    stg = ctx.enter_context(tc.tile_pool(name="stg", bufs=4))

    qt_sb = singles.tile([D, H], BF16)
    nc.sync.dma_start(out=qt_sb[:], in_=qt[:])
    id_sb = singles.tile([D, D], BF16)
    nc.sync.dma_start(out=id_sb[:], in_=ident[:])
    ones_sb = singles.tile([D, D], BF16)
    nc.vector.memset(ones_sb[:], 1.0)

    e_buf = singles.tile([128, 4, CH, E2], BF16)
    nc.gpsimd.memset(e_buf[64:128, :, :, 0:H], 0.0)
    nc.gpsimd.memset(e_buf[0:64, :, :, H:E2], 0.0)

    kx = 0  # index into xbt
    for g in range(NG):
        chunk = chunks.tile([128, CH, D], BF16)
        nc.sync.dma_start(out=chunk[:], in_=xb[:, CH * g: CH * (g + 1), :])

        xt = xtsp.tile([128, CH, D], BF16)
        if DMA_XT[g]:
            nc.scalar.dma_start(out=xt[:], in_=xbt[kx])
            kx += 1
        else:
            for q4 in range(CH // 4):
                xtb = xtp.tile([128, 4, D], BF16)
                for i in range(4):
                    c = q4 * 4 + i
                    nc.tensor.transpose(xtb[:, i, :], chunk[:, c, :], id_sb[:])
                if q4 % 2 == 0:
                    nc.vector.tensor_copy(xt[:, q4 * 4: q4 * 4 + 4, :], xtb[:])
                else:
                    nc.scalar.copy(xt[:, q4 * 4: q4 * 4 + 4, :], xtb[:])

        ltb = ltp.tile([128, CH * H], F32)
        for c in range(CH):
            nc.tensor.matmul(
                ltb[:, H * c: H * (c + 1)],
                lhsT=xt[:, c, :],
                rhs=qt_sb[:],
                start=True, stop=True,
            )

        s = g % 4
        lt3 = ltb.rearrange("p (c h) -> p c h", h=H)
        nc.scalar.activation(
            out=e_buf[0:64, s, :, 0:H], in_=lt3[0:64],
            func=mybir.ActivationFunctionType.Exp,
        )
        nc.scalar.activation(
            out=e_buf[64:128, s, :, H:E2], in_=lt3[64:128],
            func=mybir.ActivationFunctionType.Exp,
        )

        cxb = cxp.tile([128, CXW], F32)
        for c in range(CH):
            nc.tensor.matmul(
                cxb[:, E2 * c: E2 * (c + 1)],
                lhsT=chunk[:, c, :],
                rhs=e_buf[:, s, c, :],
                start=True, stop=True,
            )
        nc.tensor.matmul(
            cxb[:, 512: 512 + CH * E2],
            lhsT=ones_sb[:],
            rhs=e_buf[:, s, :, :],
            start=True, stop=True,
        )
        rr = rp.tile([128, CH * E2], F32)
        nc.vector.reciprocal_approx_fast(out=rr[:], in_=cxb[:, 512: 512 + CH * E2])
        st = stg.tile([128, CH * E2], F32)
        nc.vector.scalar_tensor_tensor(
            out=st[:], in0=cxb[:, 0: CH * E2], scalar=1.0, in1=rr[:],
            op0=mybir.AluOpType.mult, op1=mybir.AluOpType.mult,
        )
        nc.gpsimd.dma_start(out=ob[g], in_=st[:])


def _build():
    nxt = sum(DMA_XT)
    nc = bacc.Bacc("TRN2", target_bir_lowering=False, debug=False)
    xb = nc.dram_tensor("xb", [128, T // 2, D], BF16, kind="ExternalInput")
    xbt = nc.dram_tensor("xbt", [nxt, 128, CH, 128], BF16, kind="ExternalInput")
    qt = nc.dram_tensor("qt", [D, H], BF16, kind="ExternalInput")
    ident = nc.dram_tensor("ident", [D, D], BF16, kind="ExternalInput")
    ob = nc.dram_tensor("ob", [NG, 128, CH * E2], F32, kind="ExternalOutput")
    with tile.TileContext(nc) as tc:
        with ExitStack() as ctx:
            _body(ctx, tc, xb[:], xbt[:], qt[:], ident[:], ob[:])
    nc.compile()
    return nc


def get_nc():
    if "nc" not in _CACHE:
        _CACHE["nc"] = _build()
    return _CACHE["nc"]


def prep_input(others_b):
    """others[b] (N,T,D) fp32 -> (xb [(j n), tp, d], xbt [k, d, c, jn]) bf16."""
    v = others_b.reshape(N, T // 2, 2, D).astype(ml_dtypes.bfloat16)
    x = np.empty((128, T // 2, D), dtype=ml_dtypes.bfloat16)
    x[0:64] = v[:, :, 0, :]
    x[64:128] = v[:, :, 1, :]
    xs = x.reshape(128, NG, CH, D)
    sel = [g for g in range(NG) if DMA_XT[g]]
    xt = np.ascontiguousarray(xs[:, sel].transpose(1, 3, 2, 0))  # k, d, c, jn
    return x, xt


def unpack_output(ob_raw):
    """(NG, 128, CH*E2) -> (T, H, D); t = 2(CH g + c) + j, col i = 7 j + h."""
    s = ob_raw.reshape(NG, 128, CH, 2, H)          # g, d, c, j, h
    return np.ascontiguousarray(
        s.transpose(0, 2, 3, 4, 1).reshape(T, H, D)
    )


def kernel(ego=None, others=None, queries=None, _trace=False, **_unused):
    others = np.asarray(others, dtype=np.float32)
    queries = np.asarray(queries, dtype=np.float32)
    scale = float(queries.shape[-1]) ** -0.5
    qt_bf = np.ascontiguousarray(queries.T * scale).astype(ml_dtypes.bfloat16)
    eye_bf = np.eye(D, dtype=ml_dtypes.bfloat16)

    nc = get_nc()
    in_maps = []
    for b in range(B):
        x, xt = prep_input(others[b])
        in_maps.append({"xb": x, "xbt": xt, "qt": qt_bf, "ident": eye_bf})
    res = run_bass_kernel_spmd(nc, in_maps, core_ids=list(range(B)), trace=_trace)
    _CACHE["last_results"] = res
    out = np.empty((B, T, H, D), dtype=np.float32)
    for b in range(B):
        out[b] = unpack_output(res.results[b]["ob"])
    return out
